# revision 1
# baseline (speedup 1.0000x reference)
import sys

sys.path.insert(0, "/opt/trn_rl_repo")

import numpy as np

from concourse import bass, mybir, tile
from concourse import bass_utils
from concourse.masks import make_identity

B, N, K, D = 4, 16384, 32, 64
HALF = 8192
M = HALF * K            # 262144 pairs per core
CHUNK = 8192            # pairs per DMA chunk
NCHUNK = M // CHUNK     # 32
GROUP = 1024            # pairs per pipeline group (32 points x 32 nbrs)
NG = CHUNK // GROUP     # 8

TRACE = False
LAST_RESULTS = None

_BUILT = None


def _build():
    f32 = mybir.dt.float32
    f16 = mybir.dt.float16
    Copy = mybir.ActivationFunctionType.Copy
    Relu = mybir.ActivationFunctionType.Relu
    add = mybir.AluOpType.add
    mult = mybir.AluOpType.mult

    nc = bass.Bass()
    xgT_d = nc.declare_dram_parameter("xgT", [64, M], f16, False)
    relb_d = nc.declare_dram_parameter("relb", [4, M], f16, False)
    W1b_d = nc.declare_dram_parameter("W1b", [4, 64], f16, False)
    Wstk_d = nc.declare_dram_parameter("Wstk", [68, 64], f16, False)
    out_d = nc.declare_dram_parameter("out", [128, 64, 64], f32, True)

    with tile.TileContext(nc) as tc:
        frees = []

        def T(shape, dtype, name):
            t, f = tc.tile(shape, dtype, name=name)
            frees.append(f)
            return t

        W1b_sb = T([4, 64], f16, "W1b_sb")
        Wstk_sb = T([68, 64], f16, "Wstk_sb")
        acc_sb = T([64, HALF], f32, "acc_sb")
        ident = T([128, 128], f32, "ident")

        nc.sync.dma_start(W1b_sb[:, :], W1b_d[:, :])
        nc.sync.dma_start(Wstk_sb[:, :], Wstk_d[:, :])
        make_identity(nc, ident[:, :])

        with tc.tile_pool(name="xpool", bufs=2) as xpl, \
             tc.tile_pool(name="rpool", bufs=2) as rpl, \
             tc.tile_pool(name="upool", bufs=2, space="PSUM") as upl, \
             tc.tile_pool(name="wpool", bufs=2, space="PSUM") as wpl, \
             tc.tile_pool(name="spool", bufs=3) as spl, \
             tc.tile_pool(name="tpool", bufs=3) as tpl:
            for c in range(NCHUNK):
                xg_t = xpl.tile([64, CHUNK], f16, name="xg")
                rl_t = rpl.tile([4, CHUNK], f16, name="rl")
                nc.sync.dma_start(xg_t[:, :], xgT_d[:, c * CHUNK:(c + 1) * CHUNK])
                nc.sync.dma_start(rl_t[:, :], relb_d[:, c * CHUNK:(c + 1) * CHUNK])
                for g2 in range(NG):
                    g = c * NG + g2
                    lo = g2 * GROUP
                    u = upl.tile([64, GROUP], f32, name="u")
                    nc.tensor.matmul(u[:, 0:512], lhsT=W1b_sb[:, :],
                                     rhs=rl_t[:, lo:lo + 512],
                                     start=True, stop=True)
                    nc.tensor.matmul(u[:, 512:1024], lhsT=W1b_sb[:, :],
                                     rhs=rl_t[:, lo + 512:lo + GROUP],
                                     start=True, stop=True)
                    rs = spl.tile([68, GROUP], f16, name="rs")
                    nc.sync.dma_start(
                        rs[64:68, :],
                        relb_d[:, c * CHUNK + lo:c * CHUNK + lo + GROUP])
                    nc.scalar.activation(rs[0:64, :], u[:, :], Relu)
                    w = wpl.tile([64, GROUP], f32, name="w")
                    nc.tensor.matmul(w[:, 0:512], lhsT=Wstk_sb[:, :],
                                     rhs=rs[:, 0:512], start=True, stop=True)
                    nc.tensor.matmul(w[:, 512:1024], lhsT=Wstk_sb[:, :],
                                     rhs=rs[:, 512:1024], start=True, stop=True)
                    t = tpl.tile([64, 32, 32], f16, name="t")
                    nc.vector.tensor_tensor(t[:, :, :], xg_t[:, lo:lo + GROUP],
                                            w[:, :], mult)
                    nc.vector.tensor_reduce(acc_sb[:, g * 32:(g + 1) * 32],
                                            t[:, :, :],
                                            mybir.AxisListType.X, add)

        out_sb = T([128, 64, 64], f32, "out_sb")
        with tc.tile_pool(name="ppool", bufs=2, space="PSUM") as ppl:
            for tk in range(64):
                pt = ppl.tile([128, 64], f32, name="pt")
                nc.tensor.transpose(pt[:, :], acc_sb[:, tk * 128:(tk + 1) * 128],
                                    ident[0:64, 0:64])
                nc.scalar.activation(out_sb[:, tk:tk + 1, :], pt[:, :], Copy)
        nc.sync.dma_start(out_d[:, :, :], out_sb[:, :, :])
        for f in reversed(frees):
            f()

    import bass_rust
    bass_rust.move_matmul_waits_to_ldweights(nc.m)
    bass_rust.generate_event_semaphores(nc)
    mybir.codegen_inst_isa_subclasses(nc)
    return nc


def _get_nc():
    global _BUILT
    if _BUILT is None:
        _BUILT = _build()
    return _BUILT


def _prep_core(x, pos, nidx, c, W1b, Wstk):
    b, hh = c // 2, c % 2
    sl = slice(hh * HALF, (hh + 1) * HALF)
    idxh = nidx[b, sl]
    xg = x[b][idxh]                                    # [HALF, K, 64]
    rel = pos[b, sl][:, None, :] - pos[b][idxh]        # [HALF, K, 3]
    xgT = np.ascontiguousarray(xg.reshape(M, 64).T.astype(np.float16))
    relb = np.empty((4, M), np.float16)
    relb[0:3] = rel.reshape(M, 3).T
    relb[3] = 1.0
    return dict(xgT=xgT, relb=relb, W1b=W1b, Wstk=Wstk)


def kernel(x, pos, neighbor_idx, W1, b1, W2, b2):
    nc = _get_nc()
    W1b_f = np.vstack([W1, b1[None, :]]).astype(np.float32)
    Wx = W1b_f @ (0.1 * W2.astype(np.float32))         # [4, 64]
    W1b = np.ascontiguousarray(W1b_f.astype(np.float16))
    Wstk = np.ascontiguousarray(
        np.vstack([0.9 * W2, Wx]).astype(np.float16))  # [68, 64]
    in_maps = [_prep_core(x, pos, neighbor_idx, c, W1b, Wstk)
               for c in range(8)]
    global LAST_RESULTS
    res = bass_utils.run_bass_kernel_spmd(nc, in_maps, list(range(8)), trace=TRACE)
    LAST_RESULTS = res
    out = np.empty((B, N, D), np.float32)
    for c in range(8):
        b, hh = c // 2, c % 2
        r = np.asarray(res.results[c]["out"])
        out[b, hh * HALF:(hh + 1) * HALF] = r.transpose(1, 0, 2).reshape(HALF, D)
    if np.any(b2):
        for b in range(B):
            s = x[b][neighbor_idx[b]].sum(axis=1)
            out[b] += b2[None, :] * s
    return out



# revision 2
# speedup vs baseline: 1.2422x; 1.2422x over previous
import sys

sys.path.insert(0, "/opt/trn_rl_repo")

import numpy as np

from concourse import bass, mybir, tile
from concourse import bass_utils
from concourse.masks import make_identity

B, N, K, D = 4, 16384, 32, 64
HALF = 8192            # points per core
PP = HALF // 2         # 4096 point-pairs per core
M2 = PP * K            # 131072 columns (2 points per column)
PCH = 1024             # point-pairs per chunk
NCHUNK = PP // PCH     # 4
G = 512                # columns per group (one PSUM bank)
GPC = K * PCH // G     # 64 groups per chunk
NG = NCHUNK * GPC      # 256 groups total
XGB = 8                # groups per xg DMA block (4096 cols, 1 MiB)

TRACE = False
LAST_RESULTS = None
_BUILT = None


def _build():
    f32 = mybir.dt.float32
    f16 = mybir.dt.float16
    Copy = mybir.ActivationFunctionType.Copy
    Prelu = mybir.ActivationFunctionType.Prelu
    mult = mybir.AluOpType.mult

    nc = bass.Bass()
    xg2_d = nc.declare_dram_parameter("xg2", [128, M2], f16, False)
    relb_d = nc.declare_dram_parameter("relb", [NCHUNK, 4, 8, 8 * PCH], f16, False)
    W1bq_d = nc.declare_dram_parameter("W1bq", [128, 128], f16, False)
    W2d_d = nc.declare_dram_parameter("W2d", [128, 128], f16, False)
    out_d = nc.declare_dram_parameter("out", [128, PP], f16, True)

    with tile.TileContext(nc) as tc:
        frees = []

        def T(shape, dtype, name):
            t, f = tc.tile(shape, dtype, name=name)
            frees.append(f)
            return t

        W1bq_sb = T([128, 128], f16, "W1bq_sb")
        W2d_sb = T([128, 128], f16, "W2d_sb")
        ident = T([128, 128], f16, "ident")
        out_sb = T([128, PP], f16, "out_sb")

        nc.sync.dma_start(W1bq_sb[:, :], W1bq_d[:, :])
        nc.sync.dma_start(W2d_sb[:, :], W2d_d[:, :])
        make_identity(nc, ident[:, :])

        with tc.tile_pool(name="relbpool", bufs=2) as rbpl, \
             tc.tile_pool(name="xgpool", bufs=3) as xgpl, \
             tc.tile_pool(name="upool", bufs=3, space="PSUM") as upl, \
             tc.tile_pool(name="wpool", bufs=3, space="PSUM") as wpl, \
             tc.tile_pool(name="accpool", bufs=1, space="PSUM") as accpl, \
             tc.tile_pool(name="rspool", bufs=4) as rspl, \
             tc.tile_pool(name="tpool", bufs=5) as tpl:

            us, rss, ws, ts, accs = {}, {}, {}, {}, {}
            xgs = {}

            def relb_load(q):
                relb_t = rbpl.tile([128, 8 * PCH], f16, name="relb")
                for r in range(4):
                    nc.sync.dma_start(relb_t[32 * r:32 * r + 8, :],
                                      relb_d[q, r, :, :])
                return relb_t

            relbs = {0: relb_load(0)}

            def xg_load(blk):
                xg_t = xgpl.tile([128, XGB * G], f16, name="xg")
                nc.sync.dma_start(xg_t[:, :],
                                  xg2_d[:, blk * XGB * G:(blk + 1) * XGB * G])
                return xg_t

            xgs[0] = xg_load(0)

            # 5-stage software pipeline over 256 groups:
            # S0 mm1(g) | S1 prelu(g-1) | S2 mm2(g-2) | S3 mult(g-3) | S4 mm3(g-4)
            for s in range(NG + 4):
                g = s
                if g < NG:
                    q, gc = g // GPC, g % GPC
                    k, i = gc // 2, gc % 2
                    r = k // 8
                    if g % XGB == 0 and g + XGB < NG:
                        xgs[g // XGB + 1] = xg_load(g // XGB + 1)
                    if gc == 0 and q + 1 < NCHUNK:
                        relbs[q + 1] = relb_load(q + 1)
                    if gc == 0:
                        accs[q] = accpl.tile([128, PCH], f32, name="acc")
                    u = upl.tile([128, G], f32, name="u")
                    us[g] = u
                    lo = (k % 8) * PCH + i * G
                    nc.tensor.matmul(u[:, :],
                                     lhsT=W1bq_sb[32 * r:32 * r + 8, :],
                                     rhs=relbs[q][32 * r:32 * r + 8,
                                                  lo:lo + G],
                                     start=True, stop=True,
                                     tile_position=(32 * r, 0))
                g1 = s - 1
                if 0 <= g1 < NG:
                    rs = rspl.tile([128, G], f16, name="rs")
                    rss[g1] = rs
                    nc.scalar.activation(rs[:, :], us.pop(g1)[:, :], Prelu,
                                         alpha=0.1)
                g2 = s - 2
                if 0 <= g2 < NG:
                    w = wpl.tile([128, G], f32, name="w")
                    ws[g2] = w
                    nc.tensor.matmul(w[:, :], lhsT=W2d_sb[:, :],
                                     rhs=rss.pop(g2)[:, :],
                                     start=True, stop=True)
                g3 = s - 3
                if 0 <= g3 < NG:
                    t = tpl.tile([128, G], f16, name="t")
                    ts[g3] = t
                    xg_t = xgs[g3 // XGB]
                    xlo = (g3 % XGB) * G
                    nc.vector.tensor_tensor(t[:, :], ws.pop(g3)[:, :],
                                            xg_t[:, xlo:xlo + G], mult)
                g4 = s - 4
                if 0 <= g4 < NG:
                    q4, gc4 = g4 // GPC, g4 % GPC
                    k4, i4 = gc4 // 2, gc4 % 2
                    nc.tensor.matmul(accs[q4][:, i4 * G:(i4 + 1) * G],
                                     lhsT=ident[:, :], rhs=ts.pop(g4)[:, :],
                                     start=(k4 == 0), stop=(k4 == K - 1))
                    if gc4 == GPC - 1:
                        nc.scalar.activation(
                            out_sb[:, q4 * PCH:(q4 + 1) * PCH],
                            accs.pop(q4)[:, :], Copy)
        nc.sync.dma_start(out_d[:, :], out_sb[:, :])
        for f in reversed(frees):
            f()

    import bass_rust
    bass_rust.move_matmul_waits_to_ldweights(nc.m)
    bass_rust.generate_event_semaphores(nc)
    mybir.codegen_inst_isa_subclasses(nc)
    return nc


def _get_nc():
    global _BUILT
    if _BUILT is None:
        _BUILT = _build()
    return _BUILT


def _prep_core(x16, pos, nidx, c, W1bq, W2d):
    b, hh = c // 2, c % 2
    sl = slice(hh * HALF, (hh + 1) * HALF)
    idxh = nidx[b, sl]                                  # [HALF, K]
    xg = x16[b][idxh]                                   # [HALF, K, 64] f16
    rel = (pos[b, sl][:, None, :] - pos[b][idxh]).astype(np.float16)

    xgA = xg[0::2].reshape(NCHUNK, PCH, K, D).transpose(0, 2, 1, 3)
    xgB = xg[1::2].reshape(NCHUNK, PCH, K, D).transpose(0, 2, 1, 3)
    xg2 = np.empty((128, M2), np.float16)
    xg2[0:64] = xgA.reshape(M2, D).T
    xg2[64:128] = xgB.reshape(M2, D).T

    rb = np.empty((8, M2), np.float16)
    relA = rel[0::2].reshape(NCHUNK, PCH, K, 3).transpose(0, 2, 1, 3)
    relB = rel[1::2].reshape(NCHUNK, PCH, K, 3).transpose(0, 2, 1, 3)
    rb[0:3] = relA.reshape(M2, 3).T
    rb[3] = 1.0
    rb[4:7] = relB.reshape(M2, 3).T
    rb[7] = 1.0
    relb = np.ascontiguousarray(
        rb.reshape(8, NCHUNK, 4, 8 * PCH).transpose(1, 2, 0, 3))
    return dict(xg2=np.ascontiguousarray(xg2), relb=relb,
                W1bq=W1bq, W2d=W2d)


def kernel(x, pos, neighbor_idx, W1, b1, W2, b2):
    nc = _get_nc()
    W1b = np.vstack([W1, b1[None, :]]).astype(np.float16)   # [4, 64]
    W1bq = np.zeros((128, 128), np.float16)
    for r in range(4):
        W1bq[32 * r:32 * r + 4, 0:64] = W1b
        W1bq[32 * r + 4:32 * r + 8, 64:128] = W1b
    W2d = np.zeros((128, 128), np.float16)
    W2f = W2.astype(np.float16)
    W2d[0:64, 0:64] = W2f
    W2d[64:128, 64:128] = W2f

    x16 = x.astype(np.float16)
    in_maps = [_prep_core(x16, pos, neighbor_idx, c, W1bq, W2d)
               for c in range(8)]
    global LAST_RESULTS
    res = bass_utils.run_bass_kernel_spmd(nc, in_maps, list(range(8)),
                                          trace=TRACE)
    LAST_RESULTS = res
    out = np.empty((B, N, D), np.float32)
    for c in range(8):
        b, hh = c // 2, c % 2
        r = np.asarray(res.results[c]["out"])               # [128, PP] f16
        half = r.T.reshape(PP, 2, D).reshape(HALF, D)
        out[b, hh * HALF:(hh + 1) * HALF] = half.astype(np.float32)
    if np.any(b2):
        for b in range(B):
            s = x[b][neighbor_idx[b]].sum(axis=1)
            out[b] += b2[None, :] * s
    return out


# revision 3
# speedup vs baseline: 1.2533x; 1.0089x over previous
import sys

sys.path.insert(0, "/opt/trn_rl_repo")

import numpy as np

from concourse import bass, mybir, tile
from concourse import bass_utils
from concourse.masks import make_identity

B, N, K, D = 4, 16384, 32, 64
HALF = 8192            # points per core
PP = HALF // 2         # 4096 point-pairs per core
M2 = PP * K            # 131072 columns (2 points per column)
PCH = 1024             # point-pairs per chunk
NCHUNK = PP // PCH     # 4
G = 512                # columns per group (one PSUM bank)
GPC = K * PCH // G     # 64 groups per chunk
NG = NCHUNK * GPC      # 256 groups total
XGB = 16               # groups per xg DMA block (8192 cols, 2 MiB)

TRACE = False
LAST_RESULTS = None
_BUILT = None


def _build():
    f32 = mybir.dt.float32
    f16 = mybir.dt.float16
    Copy = mybir.ActivationFunctionType.Copy
    Prelu = mybir.ActivationFunctionType.Prelu
    mult = mybir.AluOpType.mult

    nc = bass.Bass()
    xg2_d = nc.declare_dram_parameter("xg2", [128, M2], f16, False)
    relb_d = nc.declare_dram_parameter("relb", [NCHUNK, 4, 8, 8 * PCH], f16, False)
    W1bq_d = nc.declare_dram_parameter("W1bq", [128, 128], f16, False)
    W2d_d = nc.declare_dram_parameter("W2d", [128, 128], f16, False)
    out_d = nc.declare_dram_parameter("out", [128, PP], f16, True)

    with tile.TileContext(nc) as tc:
        frees = []

        def T(shape, dtype, name):
            t, f = tc.tile(shape, dtype, name=name)
            frees.append(f)
            return t

        W1bq_sb = T([128, 128], f16, "W1bq_sb")
        W2d_sb = T([128, 128], f16, "W2d_sb")
        ident = T([128, 128], f16, "ident")
        out_sb = T([128, PP], f16, "out_sb")

        nc.sync.dma_start(W1bq_sb[:, :], W1bq_d[:, :])
        nc.sync.dma_start(W2d_sb[:, :], W2d_d[:, :])
        make_identity(nc, ident[:, :])

        with tc.tile_pool(name="relbpool", bufs=2) as rbpl, \
             tc.tile_pool(name="xgpool", bufs=3) as xgpl, \
             tc.tile_pool(name="upool", bufs=3, space="PSUM") as upl, \
             tc.tile_pool(name="wpool", bufs=3, space="PSUM") as wpl, \
             tc.tile_pool(name="accpool", bufs=1, space="PSUM") as accpl, \
             tc.tile_pool(name="rspool", bufs=4) as rspl, \
             tc.tile_pool(name="tpool", bufs=5) as tpl:

            us, rss, ws, ts, accs = {}, {}, {}, {}, {}
            xgs = {}

            def relb_load(q):
                relb_t = rbpl.tile([128, 8 * PCH], f16, name="relb")
                for r in range(4):
                    nc.sync.dma_start(relb_t[32 * r:32 * r + 8, :],
                                      relb_d[q, r, :, :])
                return relb_t

            relbs = {0: relb_load(0)}

            def xg_load(blk):
                xg_t = xgpl.tile([128, XGB * G], f16, name="xg")
                nc.sync.dma_start(xg_t[:, :],
                                  xg2_d[:, blk * XGB * G:(blk + 1) * XGB * G])
                return xg_t

            xgs[0] = xg_load(0)

            # 5-stage software pipeline over 256 groups:
            # S0 mm1(g) | S1 prelu(g-1) | S2 mm2(g-2) | S3 mult(g-3) | S4 mm3(g-4)
            for s in range(NG + 4):
                g = s
                if g < NG:
                    q, gc = g // GPC, g % GPC
                    k, i = gc // 2, gc % 2
                    r = k // 8
                    if g % XGB == 0 and g + XGB < NG:
                        xgs[g // XGB + 1] = xg_load(g // XGB + 1)
                    if gc == 0 and q + 1 < NCHUNK:
                        relbs[q + 1] = relb_load(q + 1)
                    if gc == 0:
                        accs[q] = accpl.tile([128, PCH], f32, name="acc")
                    u = upl.tile([128, G], f32, name="u")
                    us[g] = u
                    lo = (k % 8) * PCH + i * G
                    nc.tensor.matmul(u[:, :],
                                     lhsT=W1bq_sb[32 * r:32 * r + 8, :],
                                     rhs=relbs[q][32 * r:32 * r + 8,
                                                  lo:lo + G],
                                     start=True, stop=True,
                                     tile_position=(32 * r, 0))
                g1 = s - 1
                if 0 <= g1 < NG:
                    rs = rspl.tile([128, G], f16, name="rs")
                    rss[g1] = rs
                    nc.scalar.activation(rs[:, :], us.pop(g1)[:, :], Prelu,
                                         alpha=0.1)
                g2 = s - 2
                if 0 <= g2 < NG:
                    w = wpl.tile([128, G], f32, name="w")
                    ws[g2] = w
                    nc.tensor.matmul(w[:, :], lhsT=W2d_sb[:, :],
                                     rhs=rss.pop(g2)[:, :],
                                     start=True, stop=True)
                g3 = s - 3
                if 0 <= g3 < NG:
                    t = tpl.tile([128, G], f16, name="t")
                    ts[g3] = t
                    xg_t = xgs[g3 // XGB]
                    xlo = (g3 % XGB) * G
                    nc.vector.tensor_tensor(t[:, :], ws.pop(g3)[:, :],
                                            xg_t[:, xlo:xlo + G], mult)
                g4 = s - 4
                if 0 <= g4 < NG:
                    q4, gc4 = g4 // GPC, g4 % GPC
                    k4, i4 = gc4 // 2, gc4 % 2
                    nc.tensor.matmul(accs[q4][:, i4 * G:(i4 + 1) * G],
                                     lhsT=ident[:, :], rhs=ts.pop(g4)[:, :],
                                     start=(k4 == 0), stop=(k4 == K - 1))
                    if gc4 == GPC - 1:
                        nc.scalar.activation(
                            out_sb[:, q4 * PCH:(q4 + 1) * PCH],
                            accs.pop(q4)[:, :], Copy)
                        nc.sync.dma_start(
                            out_d[:, q4 * PCH:(q4 + 1) * PCH],
                            out_sb[:, q4 * PCH:(q4 + 1) * PCH])
        for f in reversed(frees):
            f()

    import bass_rust
    bass_rust.move_matmul_waits_to_ldweights(nc.m)
    bass_rust.generate_event_semaphores(nc)
    mybir.codegen_inst_isa_subclasses(nc)
    return nc


def _get_nc():
    global _BUILT
    if _BUILT is None:
        _BUILT = _build()
    return _BUILT


def _prep_core(x16, pos, nidx, c, W1bq, W2d):
    b, hh = c // 2, c % 2
    sl = slice(hh * HALF, (hh + 1) * HALF)
    idxh = nidx[b, sl]                                  # [HALF, K]
    xg = x16[b][idxh]                                   # [HALF, K, 64] f16
    rel = (pos[b, sl][:, None, :] - pos[b][idxh]).astype(np.float16)

    xgA = xg[0::2].reshape(NCHUNK, PCH, K, D).transpose(0, 2, 1, 3)
    xgB = xg[1::2].reshape(NCHUNK, PCH, K, D).transpose(0, 2, 1, 3)
    xg2 = np.empty((128, M2), np.float16)
    xg2[0:64] = xgA.reshape(M2, D).T
    xg2[64:128] = xgB.reshape(M2, D).T

    rb = np.empty((8, M2), np.float16)
    relA = rel[0::2].reshape(NCHUNK, PCH, K, 3).transpose(0, 2, 1, 3)
    relB = rel[1::2].reshape(NCHUNK, PCH, K, 3).transpose(0, 2, 1, 3)
    rb[0:3] = relA.reshape(M2, 3).T
    rb[3] = 1.0
    rb[4:7] = relB.reshape(M2, 3).T
    rb[7] = 1.0
    relb = np.ascontiguousarray(
        rb.reshape(8, NCHUNK, 4, 8 * PCH).transpose(1, 2, 0, 3))
    return dict(xg2=np.ascontiguousarray(xg2), relb=relb,
                W1bq=W1bq, W2d=W2d)


def kernel(x, pos, neighbor_idx, W1, b1, W2, b2):
    nc = _get_nc()
    W1b = np.vstack([W1, b1[None, :]]).astype(np.float16)   # [4, 64]
    W1bq = np.zeros((128, 128), np.float16)
    for r in range(4):
        W1bq[32 * r:32 * r + 4, 0:64] = W1b
        W1bq[32 * r + 4:32 * r + 8, 64:128] = W1b
    W2d = np.zeros((128, 128), np.float16)
    W2f = W2.astype(np.float16)
    W2d[0:64, 0:64] = W2f
    W2d[64:128, 64:128] = W2f

    x16 = x.astype(np.float16)
    in_maps = [_prep_core(x16, pos, neighbor_idx, c, W1bq, W2d)
               for c in range(8)]
    global LAST_RESULTS
    res = bass_utils.run_bass_kernel_spmd(nc, in_maps, list(range(8)),
                                          trace=TRACE)
    LAST_RESULTS = res
    out = np.empty((B, N, D), np.float32)
    for c in range(8):
        b, hh = c // 2, c % 2
        r = np.asarray(res.results[c]["out"])               # [128, PP] f16
        half = r.T.reshape(PP, 2, D).reshape(HALF, D)
        out[b, hh * HALF:(hh + 1) * HALF] = half.astype(np.float32)
    if np.any(b2):
        for b in range(B):
            s = x[b][neighbor_idx[b]].sum(axis=1)
            out[b] += b2[None, :] * s
    return out


# revision 4
# speedup vs baseline: 1.3152x; 1.0494x over previous
import sys

sys.path.insert(0, "/opt/trn_rl_repo")

import numpy as np

from concourse import bass, mybir, tile
from concourse import bass_utils
from concourse.masks import make_identity

B, N, K, D = 4, 16384, 32, 64
HALF = 8192            # points per core
PP = HALF // 2         # 4096 point-pairs per core
M2 = PP * K            # 131072 columns (2 points per column)
PCH = 1024             # point-pairs per chunk
NCHUNK = PP // PCH     # 4
G = 512                # columns per group (one PSUM bank)
GPC = K * PCH // G     # 64 groups per chunk
NG = NCHUNK * GPC      # 256 groups total
XGB = 16               # groups per xg DMA block (8192 cols, 2 MiB)

TRACE = False
LAST_RESULTS = None
_BUILT = None


def _build():
    f32 = mybir.dt.float32
    f16 = mybir.dt.float16
    Copy = mybir.ActivationFunctionType.Copy
    Prelu = mybir.ActivationFunctionType.Prelu
    mult = mybir.AluOpType.mult

    nc = bass.Bass()
    xg2_d = nc.declare_dram_parameter("xg2", [128, M2], f16, False)
    relb_d = nc.declare_dram_parameter("relb", [NCHUNK, 4, 8, 8 * PCH], f16, False)
    W1bq_d = nc.declare_dram_parameter("W1bq", [128, 128], f16, False)
    W2d_d = nc.declare_dram_parameter("W2d", [128, 128], f16, False)
    out_d = nc.declare_dram_parameter("out", [128, PP], f16, True)

    with tile.TileContext(nc) as tc:
        frees = []

        def T(shape, dtype, name):
            t, f = tc.tile(shape, dtype, name=name)
            frees.append(f)
            return t

        W1bq_sb = T([128, 128], f16, "W1bq_sb")
        W2d_sb = T([128, 128], f16, "W2d_sb")
        ident = T([128, 128], f16, "ident")
        out_sb = T([128, PP], f16, "out_sb")

        nc.sync.dma_start(W1bq_sb[:, :], W1bq_d[:, :])
        nc.sync.dma_start(W2d_sb[:, :], W2d_d[:, :])
        make_identity(nc, ident[:, :])

        with tc.tile_pool(name="relbpool", bufs=2) as rbpl, \
             tc.tile_pool(name="xgpool", bufs=4) as xgpl, \
             tc.tile_pool(name="upool", bufs=3, space="PSUM") as upl, \
             tc.tile_pool(name="wpool", bufs=3, space="PSUM") as wpl, \
             tc.tile_pool(name="accpool", bufs=1, space="PSUM") as accpl, \
             tc.tile_pool(name="rspool", bufs=4) as rspl, \
             tc.tile_pool(name="tpool", bufs=5) as tpl:

            us, rss, ws, ts, accs = {}, {}, {}, {}, {}
            xgs = {}

            def relb_load(q):
                relb_t = rbpl.tile([128, 8 * PCH], f16, name="relb")
                for r in range(4):
                    nc.sync.dma_start(relb_t[32 * r:32 * r + 8, :],
                                      relb_d[q, r, :, :])
                return relb_t

            relbs = {0: relb_load(0)}

            def xg_load(blk):
                xg_t = xgpl.tile([128, XGB * G], f16, name="xg")
                nc.sync.dma_start(xg_t[:, :],
                                  xg2_d[:, blk * XGB * G:(blk + 1) * XGB * G])
                return xg_t

            xgs[0] = xg_load(0)

            # 5-stage software pipeline over 256 groups:
            # S0 mm1(g) | S1 prelu(g-1) | S2 mm2(g-2) | S3 mult(g-3) | S4 mm3(g-4)
            for s in range(NG + 4):
                g = s
                if g < NG:
                    q, gc = g // GPC, g % GPC
                    k, i = gc // 2, gc % 2
                    r = k // 8
                    if g % XGB == 0 and g + XGB < NG:
                        xgs[g // XGB + 1] = xg_load(g // XGB + 1)
                    if gc == 0 and q + 1 < NCHUNK:
                        relbs[q + 1] = relb_load(q + 1)
                    if gc == 0:
                        accs[q] = accpl.tile([128, PCH], f32, name="acc")
                    u = upl.tile([128, G], f32, name="u")
                    us[g] = u
                    lo = (k % 8) * PCH + i * G
                    nc.tensor.matmul(u[:, :],
                                     lhsT=W1bq_sb[32 * r:32 * r + 8, :],
                                     rhs=relbs[q][32 * r:32 * r + 8,
                                                  lo:lo + G],
                                     start=True, stop=True,
                                     tile_position=(32 * r, 0))
                g1 = s - 1
                if 0 <= g1 < NG:
                    rs = rspl.tile([128, G], f16, name="rs")
                    rss[g1] = rs
                    nc.scalar.activation(rs[:, :], us.pop(g1)[:, :], Prelu,
                                         alpha=0.1)
                g2 = s - 2
                if 0 <= g2 < NG:
                    w = wpl.tile([128, G], f32, name="w")
                    ws[g2] = w
                    nc.tensor.matmul(w[:, :], lhsT=W2d_sb[:, :],
                                     rhs=rss.pop(g2)[:, :],
                                     start=True, stop=True)
                g3 = s - 3
                if 0 <= g3 < NG:
                    t = tpl.tile([128, G], f16, name="t")
                    ts[g3] = t
                    xg_t = xgs[g3 // XGB]
                    xlo = (g3 % XGB) * G
                    nc.vector.tensor_tensor(t[:, :], ws.pop(g3)[:, :],
                                            xg_t[:, xlo:xlo + G], mult)
                g4 = s - 4
                if 0 <= g4 < NG:
                    q4, gc4 = g4 // GPC, g4 % GPC
                    k4, i4 = gc4 // 2, gc4 % 2
                    nc.tensor.matmul(accs[q4][:, i4 * G:(i4 + 1) * G],
                                     lhsT=ident[:, :], rhs=ts.pop(g4)[:, :],
                                     start=(k4 == 0), stop=(k4 == K - 1))
                    if gc4 == GPC - 1:
                        nc.vector.tensor_copy(
                            out_sb[:, q4 * PCH:(q4 + 1) * PCH],
                            accs.pop(q4)[:, :])
                        nc.sync.dma_start(
                            out_d[:, q4 * PCH:(q4 + 1) * PCH],
                            out_sb[:, q4 * PCH:(q4 + 1) * PCH])
        for f in reversed(frees):
            f()

    import bass_rust
    bass_rust.move_matmul_waits_to_ldweights(nc.m)
    bass_rust.generate_event_semaphores(nc)
    mybir.codegen_inst_isa_subclasses(nc)
    return nc


def _get_nc():
    global _BUILT
    if _BUILT is None:
        _BUILT = _build()
    return _BUILT


def _prep_core(x16, pos, nidx, c, W1bq, W2d):
    b, hh = c // 2, c % 2
    sl = slice(hh * HALF, (hh + 1) * HALF)
    idxh = nidx[b, sl]                                  # [HALF, K]
    xg = x16[b][idxh]                                   # [HALF, K, 64] f16
    rel = (pos[b, sl][:, None, :] - pos[b][idxh]).astype(np.float16)

    xgA = xg[0::2].reshape(NCHUNK, PCH, K, D).transpose(0, 2, 1, 3)
    xgB = xg[1::2].reshape(NCHUNK, PCH, K, D).transpose(0, 2, 1, 3)
    xg2 = np.empty((128, M2), np.float16)
    xg2[0:64] = xgA.reshape(M2, D).T
    xg2[64:128] = xgB.reshape(M2, D).T

    rb = np.empty((8, M2), np.float16)
    relA = rel[0::2].reshape(NCHUNK, PCH, K, 3).transpose(0, 2, 1, 3)
    relB = rel[1::2].reshape(NCHUNK, PCH, K, 3).transpose(0, 2, 1, 3)
    rb[0:3] = relA.reshape(M2, 3).T
    rb[3] = 1.0
    rb[4:7] = relB.reshape(M2, 3).T
    rb[7] = 1.0
    relb = np.ascontiguousarray(
        rb.reshape(8, NCHUNK, 4, 8 * PCH).transpose(1, 2, 0, 3))
    return dict(xg2=np.ascontiguousarray(xg2), relb=relb,
                W1bq=W1bq, W2d=W2d)


def kernel(x, pos, neighbor_idx, W1, b1, W2, b2):
    nc = _get_nc()
    W1b = np.vstack([W1, b1[None, :]]).astype(np.float16)   # [4, 64]
    W1bq = np.zeros((128, 128), np.float16)
    for r in range(4):
        W1bq[32 * r:32 * r + 4, 0:64] = W1b
        W1bq[32 * r + 4:32 * r + 8, 64:128] = W1b
    W2d = np.zeros((128, 128), np.float16)
    W2f = W2.astype(np.float16)
    W2d[0:64, 0:64] = W2f
    W2d[64:128, 64:128] = W2f

    x16 = x.astype(np.float16)
    in_maps = [_prep_core(x16, pos, neighbor_idx, c, W1bq, W2d)
               for c in range(8)]
    global LAST_RESULTS
    res = bass_utils.run_bass_kernel_spmd(nc, in_maps, list(range(8)),
                                          trace=TRACE)
    LAST_RESULTS = res
    out = np.empty((B, N, D), np.float32)
    for c in range(8):
        b, hh = c // 2, c % 2
        r = np.asarray(res.results[c]["out"])               # [128, PP] f16
        half = r.T.reshape(PP, 2, D).reshape(HALF, D)
        out[b, hh * HALF:(hh + 1) * HALF] = half.astype(np.float32)
    if np.any(b2):
        for b in range(B):
            s = x[b][neighbor_idx[b]].sum(axis=1)
            out[b] += b2[None, :] * s
    return out
